# revision 13
# baseline (speedup 1.0000x reference)
"""Distributed attention kernel for 8 TRN2 NeuronCores.

Sharding: tensor-parallel over heads (2 heads/core, Megatron column split of
w_qkv), attention computed per-core for its heads over all batches. The
output projection is sliced over its CONTRACTION dim: each core multiplies
its own 128 attention-output dims against its 128 rows of w_proj for ALL
tokens, emitting a bf16 partial [DIM, T]; the host sums the 8 partials and
adds the bias. No collectives — cores are fully independent.

Layout: everything is kept transposed (d on partitions) so that
  - scores come out as S^T (keys on partitions, queries on free axis),
  - softmax needs no max subtraction (logits ~ N(0,1)),
  - the two heads run as row/col-tiled concurrent matmul pairs using the full
    128-wide PE array,
  - the projection consumes the transposed attention output directly with the
    per-core w_proj row-slice stationary (one LDWEIGHTS per 1024 tokens).
Compute dtype is bf16 with f32 PSUM accumulation.

The build is software-pipelined: QKV for batch b+1, the second projection
half of batch b-1, and (after the in-batch normalization at qi==1) the first
projection half of batch b are interleaved into attention(b)'s inner loop as
filler units so the TensorEngine never idles long enough for the HAM clock
gate to throttle it. Softmax denominators are accumulated on the VectorEngine
(two bf16 accumulators per strip), reduced across partitions by a ones-matmul,
batched through DRAM so one 128-lane DVE reciprocal serves four strips, and
broadcast back by a partition-stride-0 DMA. PSUM->SBUF projection copies
alternate between the Scalar and Vector engines to balance queue load.
"""

import os
import sys

import numpy as np

for _p in ("/opt/trn_rl_repo", os.path.expanduser("~/.axon_site/_ro/trn_rl_repo")):
    if os.path.isdir(_p) and _p not in sys.path:
        sys.path.insert(0, _p)

import ml_dtypes  # noqa: E402

import concourse.bass as bass  # noqa: E402
from concourse import bacc, bass_isa, mybir  # noqa: E402
import concourse.tile as tile  # noqa: E402
from concourse.bass_utils import run_bass_kernel_spmd  # noqa: E402

B, N, DIM, H = 4, 2048, 1024, 16
HD = DIM // H            # 64 head dim
NCORES = 8
HPC = H // NCORES        # 2 heads per core
HC = HPC * HD            # 128 head-cols per core
T = B * N                # 8192 tokens
SCALE = HD ** -0.5

BF16 = mybir.dt.bfloat16
F32 = mybir.dt.float32
EXP = mybir.ActivationFunctionType.Exp

LAST_RESULTS = None  # BassKernelResults of the most recent run (for test.py)


def _build():
    nc = bacc.Bacc(num_devices=NCORES)

    x_t = nc.declare_dram_parameter("x_t", [DIM, T], BF16, isOutput=False)
    w_c = nc.declare_dram_parameter("w_c", [DIM, 3 * HC], BF16, isOutput=False)
    # this core's 128 rows of w_proj
    w_p = nc.declare_dram_parameter("w_p", [HC, DIM], BF16, isOutput=False)
    # bf16 partial projection; host sums the 8 cores' partials + bias
    out3 = nc.declare_dram_parameter("out3", [DIM, T], BF16, isOutput=True)

    with tile.TileContext(nc) as tc:
        with (
            tc.tile_pool(name="persist", bufs=1) as persist,
            tc.tile_pool(name="xin", bufs=3) as xin,
            tc.tile_pool(name="work", bufs=3) as work,
            tc.tile_pool(name="ps_mm", bufs=2, space="PSUM") as ps_mm,
            tc.tile_pool(name="ps_s", bufs=2, space="PSUM") as ps_s,
            tc.tile_pool(name="ps_o", bufs=2, space="PSUM") as ps_o,
            tc.tile_pool(name="dram", bufs=1, space="DRAM") as dram,
        ):
            # ---- persistent SBUF tensors ----
            wqkv_sb = persist.tile([128, 8, 3 * HC], BF16)
            wproj_sb = persist.tile([128, 8, 128], BF16)   # [dim, od, odcol]
            ones_sb = persist.tile([128, 1], BF16)
            QT = persist.tile([128, T], BF16)
            KT = persist.tile([128, T], BF16)
            Vp = persist.tile([128, B, 16, HPC, HD], BF16)
            attnT = persist.tile([128, T], BF16)   # rows = h*64 + d

            # ---- DRAM staging for softmax denominators ----
            den_d = dram.tile([B, 2 * 4, 512], BF16)    # idx = qi*2 + h
            rden_d = dram.tile([B, 2 * 4, 512], BF16)

            for k in range(8):
                nc.sync.dma_start(wqkv_sb[:, k, :], w_c[k * 128:(k + 1) * 128, :])
            nc.sync.dma_start(wproj_sb, w_p[:, :])
            nc.vector.memset(ones_sb, 1.0)

            # ---- phase builders ----
            def qkv_chunk_units(tq, kv_first=False):
                """1024-token QKV chunk as a list of filler closures."""
                st = {}

                def u_dma():
                    xt = xin.tile([128, 8, 1024], BF16, tag="xt", name=f"xt{tq}")
                    for k in range(8):
                        nc.sync.dma_start(
                            xt[:, k, :],
                            x_t[k * 128:(k + 1) * 128, tq * 1024:(tq + 1) * 1024],
                        )
                    st["xt"] = xt

                def mk_qk(m, nh, half):
                    def u():
                        xt = st["xt"]
                        if half == 0:
                            st[(m, nh)] = ps_mm.tile(
                                [128, 512], F32, tag="mm", name=f"pqk{tq}{m}{nh}"
                            )
                        pmm = st[(m, nh)]
                        for k in range(4 * half, 4 * half + 4):
                            nc.tensor.matmul(
                                pmm,
                                wqkv_sb[:, k, m * 128:(m + 1) * 128],
                                xt[:, k, nh * 512:(nh + 1) * 512],
                                start=(k == 0),
                                stop=(k == 7),
                            )
                        if half == 1:
                            dst = QT if m == 0 else KT
                            nc.vector.tensor_copy(
                                dst[:, tq * 1024 + nh * 512:
                                    tq * 1024 + (nh + 1) * 512],
                                pmm,
                            )
                    return u

                def mk_v(st_idx):
                    def u():
                        xt = st["xt"]
                        pv = ps_mm.tile([128, 128], F32, tag="mm", name=f"pv{tq}{st_idx}")
                        for k in range(8):
                            nc.tensor.matmul(
                                pv,
                                xt[:, k, st_idx * 128:(st_idx + 1) * 128],
                                wqkv_sb[:, k, 2 * HC:3 * HC],
                                start=(k == 0),
                                stop=(k == 7),
                            )
                        gt = tq * 8 + st_idx
                        b, lt = gt // 16, gt % 16
                        nc.vector.tensor_copy(Vp[:, b, lt, :, :], pv)
                    return u

                if kv_first:
                    # K and V as early as possible (attention consumes them
                    # incrementally along the kj axis); Q strips last
                    units = [u_dma]
                    for nh in range(2):
                        units += [mk_qk(1, nh, 0), mk_qk(1, nh, 1)]
                    for st_idx in range(8):
                        units.append(mk_v(st_idx))
                    for nh in range(2):
                        units += [mk_qk(0, nh, 0), mk_qk(0, nh, 1)]
                else:
                    units = [u_dma]
                    for m in range(2):
                        for nh in range(2):
                            units.append(mk_qk(m, nh, 0))
                            units.append(mk_qk(m, nh, 1))
                    for st_idx in range(8):
                        units.append(mk_v(st_idx))
                return units

            def proj_half_units(b, hf):
                """Partial projection of batch b's qi strips 2hf, 2hf+1.

                Contraction is only over this core's 128 attn dims, so each
                od block is one LDWEIGHTS + two N=512 matmuls. PSUM->SBUF
                copies alternate Scalar/Vector to balance the queues.
                """
                t0 = b * N + hf * 1024

                def mk_od(od):
                    def u():
                        ob = work.tile([128, 2, 512], BF16, tag="ob",
                                       name=f"ob{b}{hf}{od}")
                        for s in range(2):
                            pp = ps_mm.tile([128, 512], F32, tag="mm",
                                            name=f"pp{b}{hf}{od}{s}")
                            nc.tensor.matmul(
                                pp, wproj_sb[:, od, :],
                                attnT[:, t0 + s * 512:t0 + (s + 1) * 512],
                                start=True, stop=True,
                            )
                            if (od + s) % 2 == 0:
                                nc.scalar.copy(ob[:, s, :], pp)
                            else:
                                nc.vector.tensor_copy(ob[:, s, :], pp)
                        nc.sync.dma_start(
                            out3[od * 128:(od + 1) * 128, t0:t0 + 1024], ob
                        )
                    return u

                return [mk_od(od) for od in range(8)]

            def norm_half(b, half):
                """Reciprocal + broadcast-multiply for qi strips 2h, 2h+1."""
                t0 = b * N
                # view the 4x512 denominator block as [128, 16] so the DVE
                # iterative-divide runs on all 128 lanes, not 4
                src = den_d[b, 4 * half:4 * half + 4, :]
                wide = bass.AP(tensor=src.tensor, offset=src.offset,
                               ap=[[16, 128], [1, 16]])
                den_sb = work.tile([128, 16], BF16, tag="den", name=f"den{b}{half}")
                nc.sync.dma_start(den_sb, wide)
                rden_f = work.tile([128, 16], F32, tag="rdenf", name=f"rdf{b}{half}")
                nc.vector.reciprocal(rden_f, den_sb)
                rden_b = work.tile([128, 16], BF16, tag="rdenb", name=f"rdb{b}{half}")
                nc.vector.tensor_copy(rden_b, rden_f)
                dstw = rden_d[b, 4 * half:4 * half + 4, :]
                widew = bass.AP(tensor=dstw.tensor, offset=dstw.offset,
                                ap=[[16, 128], [1, 16]])
                nc.sync.dma_start(widew, rden_b)
                for qi in (2 * half, 2 * half + 1):
                    q0 = t0 + qi * 512
                    bc = work.tile([128, 512], BF16, tag="bc")
                    for h in range(HPC):
                        src = rden_d[b, qi * 2 + h, :]
                        bcast = bass.AP(tensor=src.tensor, offset=src.offset,
                                        ap=[[0, HD], [1, 512]])
                        nc.sync.dma_start(bc[h * HD:(h + 1) * HD, :], bcast)
                    nc.vector.tensor_mul(
                        attnT[:, q0:q0 + 512], attnT[:, q0:q0 + 512], bc
                    )

            # ---- minimal batch-0 prologue: x chunk 0, Q/K for kj 0-3, V 0-1
            ch0 = qkv_chunk_units(0)
            for i in (0, 1, 2, 5, 6, 9, 10):
                ch0[i]()
            ch0_rest = [ch0[i] for i in (7, 8, 11, 12, 13, 14, 15, 16, 3, 4)]

            # ---- pipelined main loop over batches ----
            for b in range(B):
                t0 = b * N
                if b == 0:
                    fillers = (ch0_rest + qkv_chunk_units(1, kv_first=True)
                               + qkv_chunk_units(2) + qkv_chunk_units(3))
                else:
                    fillers = []
                    fillers += proj_half_units(b - 1, 1)
                    if b + 1 < B:
                        fillers += qkv_chunk_units(2 * (b + 1))
                        fillers += qkv_chunk_units(2 * (b + 1) + 1)
                fillers.reverse()  # pop() from the end = original order
                n_fill = len(fillers)
                popped = 0

                # flat list of (qi, kj) steps; S^T pairs are emitted one step
                # ahead so the ACT-feeding matmul is never queued behind the
                # eS-gated V matmuls or filler work on the in-order PE queue
                steps = [(qi, kj) for qi in range(4) for kj in range(16)]
                pS_t = {}
                po_t = {}
                acc_t = {}

                def emit_S(qi, kj):
                    q0 = t0 + qi * 512
                    k0 = t0 + kj * 128
                    pS = ps_s.tile([128, 2, 512], F32, tag="s",
                                   name=f"pS{b}_{qi}_{kj}")
                    for h in range(HPC):
                        hs = h * HD
                        nc.tensor.matmul(
                            pS[:, h, :],
                            KT[hs:hs + HD, k0:k0 + 128],
                            QT[hs:hs + HD, q0:q0 + 512],
                            start=True,
                            stop=True,
                        )
                    pS_t[(qi, kj)] = pS

                emit_S(0, 0)
                for it, (qi, kj) in enumerate(steps):
                    q0 = t0 + qi * 512
                    if kj == 0:
                        po_t[qi] = ps_o.tile([128, 512], F32, tag="vo",
                                             name=f"po{b}_{qi}")
                        acc_t[qi] = [
                            work.tile([128, 2, 512], BF16, tag=f"acc{a}",
                                      name=f"acc{a}_{b}_{qi}")
                            for a in range(2)
                        ]
                    if b == 0 and it < 16:
                        target = 2 * (it + 1)
                    else:
                        target = (it + 1) * n_fill // 56
                    due = max(0, target - popped)
                    # sandwich the filler work around the two sem-gated
                    # instructions (S waiting its PSUM slot, V waiting eS) so
                    # the in-order PE queue never idles at a blocked head
                    for _ in range(due // 2):
                        if fillers:
                            fillers.pop()()
                            popped += 1
                    if it + 1 < len(steps):
                        emit_S(*steps[it + 1])
                    pS = pS_t.pop((qi, kj))
                    eS = work.tile([128, 2, 512], BF16, tag="es", bufs=8)
                    nc.scalar.activation(eS, pS, EXP, scale=SCALE)
                    for _ in range(due - due // 2):
                        if fillers:
                            fillers.pop()()
                            popped += 1
                    po, acc = po_t[qi], acc_t[qi]
                    for h in range(HPC):
                        nc.tensor.matmul(
                            po[h * HD:(h + 1) * HD, :],
                            Vp[:, b, kj, h, :],
                            eS[:, h, :],
                            start=(kj == 0),
                            stop=(kj == 15),
                        )
                    a = kj // 8
                    if kj % 8 == 0:
                        nc.vector.tensor_copy(acc[a], eS)
                    else:
                        nc.vector.tensor_add(acc[a], acc[a], eS)
                    if kj == 15:
                        # stage numerators (unnormalized, one copy, both heads)
                        nc.vector.tensor_copy(attnT[:, q0:q0 + 512], po)
                        # denominators: partition-reduce the accumulators
                        nc.vector.tensor_add(acc[0], acc[0], acc[1])
                        for h in range(HPC):
                            idx = qi * 2 + h
                            pden = ps_mm.tile([1, 512], F32, tag="mm",
                                              name=f"pden{b}{idx}")
                            nc.tensor.matmul(pden, ones_sb[:, 0:1],
                                             acc[0][:, h, :],
                                             start=True, stop=True)
                            dstage = work.tile([1, 512], BF16, tag="dst")
                            nc.vector.tensor_copy(dstage, pden)
                            nc.sync.dma_start(den_d[b, idx, :], dstage)
                        if qi == 1:
                            # first half normalized + projected while qi
                            # strips 2-3 are still computing
                            norm_half(b, 0)
                            new = proj_half_units(b, 0)
                            fillers[:0] = list(reversed(new))
                            n_fill += len(new)
                while fillers:
                    fillers.pop()()

                norm_half(b, 1)

            # ---- second projection half of the last batch ----
            for u in proj_half_units(B - 1, 1):
                u()

    nc.finalize()
    return nc


def kernel(x, w_qkv, w_proj, b_proj):
    global LAST_RESULTS
    bf16 = ml_dtypes.bfloat16

    x_t = np.ascontiguousarray(x.reshape(T, DIM).T.astype(bf16))  # [DIM, T]

    in_maps = []
    for c in range(NCORES):
        w_c = np.concatenate(
            [
                w_qkv[:, HC * c:HC * (c + 1)],
                w_qkv[:, DIM + HC * c:DIM + HC * (c + 1)],
                w_qkv[:, 2 * DIM + HC * c:2 * DIM + HC * (c + 1)],
            ],
            axis=1,
        ).astype(bf16)
        w_p = np.ascontiguousarray(
            w_proj[HC * c:HC * (c + 1), :].astype(bf16)
        )
        in_maps.append(
            {"x_t": x_t, "w_c": np.ascontiguousarray(w_c), "w_p": w_p}
        )

    nc = _build()
    LAST_RESULTS = run_bass_kernel_spmd(
        nc, in_maps, core_ids=list(range(NCORES)),
        trace=bool(os.environ.get("KERNEL_TRACE")),
    )

    # each core returns a bf16 partial projection [DIM, T]; sum + bias here
    out_T = np.zeros((DIM, T), dtype=np.float32)
    for c in range(NCORES):
        out_T += np.asarray(LAST_RESULTS.results[c]["out3"], dtype=np.float32)
    out_T += np.asarray(b_proj, dtype=np.float32)[:, None]
    return np.ascontiguousarray(out_T.T).reshape(B, N, DIM).astype(np.float32)
